# revision 52
# baseline (speedup 1.0000x reference)
"""Trainium2 Bass kernel for nn_C2BM_30537217474758 (gnn_message_passing).

Concept-bottleneck model:
  x_enc = lrelu(x @ W_enc + b_enc)                         [B, 1024]
  vals  = lrelu(einsum('bi,rio->bro', x_enc, Wv) + bv)     [B, 8, 256]
  p_root = softmax(einsum('bro,roc->brc', vals, Ws) + bs)  [B, 8, 4]
  p_root = intervene(p_root, c[:, :8], ii[:, :8])
  h     = lrelu(einsum('bp,nph->bnh', p_root.flat, W1c) + b1c)
  p_mid = softmax(einsum('bnh,nhc->bnc', h, W2c) + b2c); intervene
  y     = softmax(lrelu(p_mid.flat @ W1y + b1y) @ W2y + b2y)
  out   = concat([p_root, p_mid, y[:, None]], axis=1)      [B, 17, 4]

Strategy: pure data-parallel over 8 NeuronCores (batch shard 1024/core),
weights replicated.  The two large GEMMs (encoder and value-embedding,
~4.3 GFLOP each per core) run in fp8(e4m3) with DoubleRow perf mode (2x PE
throughput, fp32 PSUM accumulation); weights and x are pre-scaled on the
host (x*32, W*256) so fp8 quantization happens in the normal range, and
the scales are divided back out in the activation (lrelu is positively
homogeneous).  x is transposed and cast on the HOST, so the kernel does
zero on-chip transposition of x.

The scorer and mid/task propagators produce logits directly in TRANSPOSED
layout [32 = 8grp x 4card, batch] by using zero-padded block stationary
matrices, so softmax group sums become one tiny block-diagonal matmul and
the resulting probability tensor feeds the next propagator GEMM with no
transpose on the critical path.  Intervention one-hots/masks are
precomputed on the host in the same transposed layout.  Output staging
[batch, 68] is produced by small PE transposes off the critical path, and
the final DRAM output is [128, 8*68] per core, unsharded on the host.

Batch is processed in two 512-row halves so each half's softmax ->
propagator -> task tail (DVE/ACT latency chains) hides under the other
half's GEMMs.
"""

import os
import sys

try:
    import concourse  # noqa: F401
except ImportError:
    sys.path.insert(0, "/opt/trn_rl_repo")

import numpy as np
import ml_dtypes

import concourse.bacc as bacc
import concourse.tile as tile
from concourse import mybir

# ---------------- problem constants (hardcoded per contract) ----------------
B, D_IN, D_H = 8192, 2048, 1024
N_ROOT, N_MID, CARD, CHS = 8, 8, 4, 64
OV = CARD * CHS           # 256  value-embedding width per root
P_IN = N_ROOT * CARD      # 32
P_HID = 2 * P_IN          # 64
N_CORES = 8
BSH = B // N_CORES        # 1024 batch rows per core
KT_IN = D_IN // 128       # 16 contraction tiles for encoder
KT_H = D_H // 128         # 8 contraction tiles for Wv
OUTW = 17 * CARD          # 68 output cols per row

F32 = mybir.dt.float32
I32 = mybir.dt.int32
U8 = mybir.dt.uint8
BF16 = mybir.dt.bfloat16
FP8 = mybir.dt.float8e4
AF = mybir.ActivationFunctionType
ALU = mybir.AluOpType
AX = mybir.AxisListType
DR = mybir.MatmulPerfMode.DoubleRow

LRELU_ALPHA = 0.01
# host-side pre-scales so fp8 values land in the normal range
SX = 32.0                 # x and x_enc scale
SW = 256.0                # W_enc / Wv scale
# CoreSim does not implement Lrelu/Prelu; BASS_SIM_SAFE=1 swaps in Relu so
# the rest of the program can be validated in simulation.  On hardware we
# use Prelu (identical to leaky-relu via the alpha operand): it lives in
# the same activation-table set as Exp ('exp_and_others'), so the Act
# engine never reloads tables between lrelu and softmax work.
SIM_SAFE = os.environ.get("BASS_SIM_SAFE") == "1"
ACT_LRELU = AF.Relu if SIM_SAFE else AF.Prelu


def build_program():
    """Emit the per-core Bass program (identical on all 8 cores)."""
    nc = bacc.Bacc("TRN2", target_bir_lowering=False, debug=False,
                   num_devices=N_CORES)

    # ------------- DRAM I/O (all host-prepped layouts) -------------
    # xt: [p, half, chunk, kt, b] = 32*x[g*512+c*256+b, kt*128+p] in fp8
    xt_d = nc.dram_tensor("xt", [128, 2 * 2 * KT_IN * 256], FP8,
                          kind="ExternalInput")
    # wenc: [p, ht, kt, c] = 256*W_enc[kt*128+p, ht*128+c]
    wenc_d = nc.dram_tensor("wenc", [128, KT_H * KT_IN * 128], FP8,
                            kind="ExternalInput")
    # wv: [p, r, kt, oc] = 256*Wv[r, kt*128+p, oc]
    wv_d = nc.dram_tensor("wv", [128, N_ROOT * KT_H * OV], FP8,
                          kind="ExternalInput")
    # packed fp32 constants: benc(0:8) | bv(8:24) | bsT col 24 | b2cT col 25
    cstf_d = nc.dram_tensor("cstf", [128, 26], F32, kind="ExternalInput")
    # packed bf16 constants: ws_big [ch, 2r+ot, 4r+c] (cols 0:512) |
    # w2c_big [64s+h, q, 4(2q+s)+c] (512:640) | w1c+b1c [33, 4, 128]
    # (640:1152) | w1y+b1y [33, 64] (1152:1216) | w2y+b2y [65, 4]
    # (1216:1220) | ident32 (1220:1252) | g32 (1252:1284)
    cstb_d = nc.dram_tensor("cstb", [128, 1284], BF16, kind="ExternalInput")
    # transposed one-hots (bf16) and masks (u8): [4g+c | 4n+c, b];
    # cols 0:1024 = root level, 1024:2048 = mid level
    ohb_d = nc.dram_tensor("ohb", [P_IN, 2 * BSH], BF16, kind="ExternalInput")
    mb_d = nc.dram_tensor("mb", [P_IN, 2 * BSH], U8, kind="ExternalInput")
    # out: [p, bt, 68]
    out_d = nc.dram_tensor("out", [128, (BSH // 128) * OUTW], F32,
                           kind="ExternalOutput")

    with tile.TileContext(nc) as tc:
        with (
            tc.tile_pool(name="persist", bufs=1) as persist,
            tc.tile_pool(name="vals", bufs=3) as vals_pool,
            tc.tile_pool(name="tmp", bufs=2) as tmp_pool,
            tc.tile_pool(name="ps_mm", bufs=3, space="PSUM") as ps_mm,
            tc.tile_pool(name="ps_lg", bufs=1, space="PSUM") as ps_lg,
            tc.tile_pool(name="ps_sm", bufs=2, space="PSUM") as ps_sm,
        ):
            # -------- DMA order: x h0 + wenc ht0 gate the encoder ----------
            # SWDGE ring: xt halves then wv (needed from ~t+20us).
            # SP ring: wenc ht0, fp32 consts (gate the first act), rest of
            # wenc, packed bf16 consts, one-hots/masks.
            xt_sb = persist.tile([128, 2, 2, KT_IN, 256], FP8)
            xt_r = xt_d.ap().rearrange("p (g c k b) -> p g c k b",
                                       g=2, c=2, b=256)
            wenc_sb = persist.tile([128, KT_H, KT_IN, 128], FP8)
            wenc_r = wenc_d.ap().rearrange("p (h k c) -> p h k c",
                                           h=KT_H, c=128)
            for k4 in range(4):
                nc.gpsimd.dma_start(out=xt_sb[:, 0, 0, 4 * k4:4 * k4 + 4],
                                    in_=xt_r[:, 0, 0, 4 * k4:4 * k4 + 4])
                nc.sync.dma_start(out=wenc_sb[:, 0, 4 * k4:4 * k4 + 4],
                                  in_=wenc_r[:, 0, 4 * k4:4 * k4 + 4])
            nc.gpsimd.dma_start(out=xt_sb[:, 0, 1], in_=xt_r[:, 0, 1])
            nc.sync.dma_start(out=wenc_sb[:, 1], in_=wenc_r[:, 1])
            cstf_sb = persist.tile([128, 26], F32)
            nc.sync.dma_start(out=cstf_sb, in_=cstf_d.ap())
            benc_sb = cstf_sb[:, 0:8]
            bv_sb = cstf_sb[:, 8:24]
            bsT_sb = cstf_sb[0:P_IN, 24:25]
            b2cT_sb = cstf_sb[0:P_IN, 25:26]
            wv_sb = persist.tile([128, N_ROOT, KT_H, OV], FP8)
            wv_r = wv_d.ap().rearrange("p (r k o) -> p r k o",
                                       r=N_ROOT, o=OV)
            for r in range(N_ROOT):
                nc.gpsimd.dma_start(out=wv_sb[:, r], in_=wv_r[:, r])
            nc.gpsimd.dma_start(out=xt_sb[:, 1, 0], in_=xt_r[:, 1, 0])
            nc.gpsimd.dma_start(out=xt_sb[:, 1, 1], in_=xt_r[:, 1, 1])
            for ht in range(2, KT_H):
                nc.sync.dma_start(out=wenc_sb[:, ht], in_=wenc_r[:, ht])

            # packed bf16 constants (one DMA): wsb | w2cb | w1c | w1y | w2y
            # | ident | g32
            cstb_sb = persist.tile([128, 1284], BF16)
            nc.sync.dma_start(out=cstb_sb, in_=cstb_d.ap())
            wsb_sb = cstb_sb[:, 0:512].rearrange("p (q c) -> p q c", c=32)
            w2cb_sb = cstb_sb[:, 512:640].rearrange("p (q c) -> p q c", c=32)
            w1c_sb = cstb_sb[0:P_IN + 1, 640:1152].rearrange(
                "p (q m) -> p q m", m=128)
            w1y_sb = cstb_sb[0:P_IN + 1, 1152:1216]
            w2y_sb = cstb_sb[0:P_HID + 1, 1216:1220]
            ident_sb = cstb_sb[0:P_IN, 1220:1252]
            g32_sb = cstb_sb[0:P_IN, 1252:1284]

            ohb_sb = persist.tile([P_IN, 2 * BSH], BF16)
            nc.sync.dma_start(out=ohb_sb, in_=ohb_d.ap())
            mb_sb = persist.tile([P_IN, 2 * BSH], U8)
            nc.sync.dma_start(out=mb_sb, in_=mb_d.ap())

            # warm-up matmul source: memset, so PE filler never waits DMA
            wsrc_sb = persist.tile([P_IN, P_IN], BF16)
            nc.vector.memset(wsrc_sb, 1.0)

            # ---------------- persistent activations ----------------
            xenc_sb = persist.tile([128, KT_H, BSH], FP8)   # 32*x_encT
            prT_sb = persist.tile([P_IN + 1, BSH], BF16)    # row 32 = ones
            nc.vector.memset(prT_sb[P_IN:P_IN + 1, :], 1.0)
            pmT_sb = persist.tile([P_IN + 1, BSH], BF16)
            nc.vector.memset(pmT_sb[P_IN:P_IN + 1, :], 1.0)
            hyT_sb = persist.tile([P_HID + 1, BSH], BF16)   # row 64 = ones
            nc.vector.memset(hyT_sb[P_HID:P_HID + 1, :], 1.0)
            hT_sb = persist.tile([128, 4, BSH], BF16)  # [2 mids x 64h, b]
            osb = persist.tile([128, BSH // 128, OUTW], F32)

            # ---------------- encoder GEMM -> x_encT (fp8) ----------------
            def encoder_half(g, hooks=None):
                hooks = hooks or {}
                for ht in range(KT_H):
                    ps = ps_mm.tile([128, 512], F32, tag="mm")
                    for c in range(2):
                        for j in range(KT_IN // 2):
                            nc.tensor.matmul(
                                ps[:, c * 256:(c + 1) * 256],
                                wenc_sb[:, ht, 2 * j:2 * j + 2, :],
                                xt_sb[:, g, c, 2 * j:2 * j + 2, :],
                                start=(j == 0), stop=(j == KT_IN // 2 - 1),
                                perf_mode=DR, skip_group_check=(c == 1))
                    nc.scalar.activation(
                        xenc_sb[:, ht, g * 512:(g + 1) * 512], ps,
                        ACT_LRELU, bias=benc_sb[:, ht:ht + 1],
                        scale=float(SX / (SX * SW)), alpha=LRELU_ALPHA)
                    if ht in hooks:
                        hooks[ht]()

            def win(g, ch):
                """column window: absolute start, width for (half, chunk)."""
                if ch is None:
                    return g * 512, 512
                return g * 512 + ch * 256, 256

            # ------------- per-root value GEMM + scorer (one window) --------
            def vals_scorer(g, lg, ch=None, hooks=None):
                """Value embeddings + scorer over one column window;
                logitsT into lg[:, window].  The scorer for root r is
                emitted after root r+1's value GEMMs so the PE never
                stalls on the vals activation (a stall resets the PE
                pstate ramp).  hooks[r] emits extra (tail) work after
                root r's GEMMs."""
                c0, n = win(g, ch)
                lgw = lg if ch is None else lg[:, ch * 256:(ch + 1) * 256]
                hooks = hooks or {}
                vts = {}

                def scorer(r):
                    for ot in range(2):
                        nc.tensor.matmul(
                            lgw, wsb_sb[:, 2 * r + ot, :], vts[r][:, ot, :],
                            start=(r == 0 and ot == 0),
                            stop=(r == N_ROOT - 1 and ot == 1),
                            skip_group_check=(ch == 1))

                shared = [None]
                for r in range(N_ROOT):
                    vt = vals_pool.tile([128, 2, n], BF16, tag="vals")
                    vts[r] = vt
                    for ot in range(2):
                        if n == 512:
                            ps = ps_mm.tile([128, 512], F32, tag="mm")
                            reg = 0
                        else:
                            # pair two 256-wide chunks per [128,512] tile
                            reg = (2 * r + ot) % 2
                            if reg == 0:
                                shared[0] = ps_mm.tile([128, 512], F32,
                                                       tag="mm",
                                                       name="vshare")
                            ps = shared[0][:, reg * 256:(reg + 1) * 256]
                        for c in range(n // 256):
                            nc_ps = ps[:, c * 256:(c + 1) * 256] \
                                if n == 512 else ps
                            for j in range(KT_H // 2):
                                nc.tensor.matmul(
                                    nc_ps,
                                    wv_sb[:, r, 2 * j:2 * j + 2,
                                          ot * 128:(ot + 1) * 128],
                                    xenc_sb[:, 2 * j:2 * j + 2,
                                            c0 + c * 256:c0 + (c + 1) * 256],
                                    start=(j == 0), stop=(j == KT_H // 2 - 1),
                                    perf_mode=DR,
                                    skip_group_check=(c == 1 or reg == 1))
                        nc.scalar.activation(
                            vt[:, ot, :], ps, ACT_LRELU,
                            bias=bv_sb[:, 2 * r + ot:2 * r + ot + 1],
                            scale=float(1.0 / (SX * SW)), alpha=LRELU_ALPHA)
                    if r >= 1:
                        scorer(r - 1)
                    if r in hooks:
                        hooks[r]()
                scorer(N_ROOT - 1)

            # ------------- transposed softmax + intervention tail ----------
            def softmax_chain(g, lg, bias, lv, pT, ch=None, sc=1.0):
                """softmax+intervene on logitsT window of lg (PSUM);
                probs -> pT[0:32, window] (bf16)."""
                c0, n = win(g, ch)
                lgw = lg if ch is None else lg[:, ch * 256:(ch + 1) * 256]
                cols = slice(c0, c0 + n)
                pcols = slice(lv * BSH + c0, lv * BSH + c0 + n)
                e = tmp_pool.tile([P_IN, n], BF16, tag="e", bufs=3)
                nc.scalar.activation(e, lgw, AF.Exp, bias=bias, scale=sc)
                sm = ps_lg.tile([P_IN, n], F32, tag="lg", bufs=2, name="sums")
                nc.tensor.matmul(sm, g32_sb, e, start=True, stop=True)
                rcp = tmp_pool.tile([P_IN, n], F32, tag="rcp", bufs=2)
                nc.vector.reciprocal_approx_fast(rcp, sm)
                nc.vector.tensor_tensor(pT[0:P_IN, cols], e, rcp, op=ALU.mult)
                nc.vector.copy_predicated(pT[0:P_IN, cols], mb_sb[:, pcols],
                                          ohb_sb[:, pcols])

            def dve_lrelu(dst, ps, n, tag):
                if SIM_SAFE:
                    nc.vector.tensor_scalar(dst, ps, 0.0, None, op0=ALU.max)
                else:
                    t = tmp_pool.tile([128, n], BF16, tag=tag, bufs=2)
                    tt = t[0:dst.partition_size(), :]
                    nc.vector.tensor_scalar(tt, ps, LRELU_ALPHA, None,
                                            op0=ALU.mult)
                    nc.vector.tensor_tensor(dst, ps, tt, op=ALU.max)

            def mid_h_mms(g, q, ch=None, act_lrelu=False):
                c0, n = win(g, ch)
                ps = ps_mm.tile([128, n], F32, tag="mmh", bufs=2)
                nc.tensor.matmul(ps, w1c_sb[:, q, :], prT_sb[:, c0:c0 + n],
                                 start=True, stop=True)
                dst = hT_sb[:, q, c0:c0 + n]
                if act_lrelu and q % 2 == 0 and not SIM_SAFE:
                    nc.scalar.activation(dst, ps, ACT_LRELU,
                                         alpha=LRELU_ALPHA)
                else:
                    dve_lrelu(dst, ps, n, "lr")

            def mid_logit_mms(g, ml, ch=None):
                c0, n = win(g, ch)
                mlw = ml if ch is None else ml[:, ch * 256:(ch + 1) * 256]
                for q in range(4):
                    nc.tensor.matmul(
                        mlw, w2cb_sb[:, q, :], hT_sb[:, q, c0:c0 + n],
                        start=(q == 0), stop=(q == 3),
                        skip_group_check=(ch == 1))

            def bts_of(g, ch):
                if ch is None:
                    return list(range(4 * g, 4 * g + 4))
                return [4 * g + 2 * ch, 4 * g + 2 * ch + 1]

            def task_mms(g, ch=None, act_lrelu=False):
                c0, n = win(g, ch)
                if n == 512:
                    ps = ps_mm.tile([P_HID, 512], F32, tag="mm")
                else:
                    ps = ps_mm.tile([P_HID, 256], F32, tag="mmh", bufs=2)
                nc.tensor.matmul(ps, w1y_sb, pmT_sb[:, c0:c0 + n],
                                 start=True, stop=True)
                dst = hyT_sb[0:P_HID, c0:c0 + n]
                if act_lrelu and not SIM_SAFE:
                    nc.scalar.activation(dst, ps, ACT_LRELU,
                                         alpha=LRELU_ALPHA)
                else:
                    dve_lrelu(dst, ps, n, "lry")

            def y_tail(g, ch=None, name="yl"):
                bts = bts_of(g, ch)
                nb = len(bts)
                yl = ps_sm.tile([128, 4 * nb], F32, tag="ptr", bufs=1,
                                name=name)
                for i, bt in enumerate(bts):
                    nc.tensor.matmul(
                        yl[:, i * 4:(i + 1) * 4],
                        hyT_sb[:, bt * 128:(bt + 1) * 128], w2y_sb,
                        start=True, stop=True, skip_group_check=True)
                e4 = tmp_pool.tile([128, 4 * nb], F32, tag="e4")
                nc.scalar.activation(e4, yl[:, 0:4 * nb], AF.Exp)
                s1 = tmp_pool.tile([128, nb], F32, tag="s1")
                nc.vector.reduce_sum(
                    s1, e4.rearrange("p (b c) -> p b c", c=CARD), axis=AX.X)
                r1 = tmp_pool.tile([128, nb], F32, tag="r1")
                nc.vector.reciprocal(r1, s1)
                nc.vector.tensor_tensor(
                    osb[:, bts[0]:bts[0] + nb, 16 * CARD:17 * CARD],
                    e4.rearrange("p (b c) -> p b c", c=CARD),
                    r1.unsqueeze(2).broadcast_to([128, nb, CARD]),
                    op=ALU.mult)

            def osb_transposes(g, pT, lv, ch=None):
                """pT[0:32, window] -> osb[:, bt, lv*32:(lv+1)*32]."""
                for bt in bts_of(g, ch):
                    trp = ps_sm.tile([128, P_IN], BF16, tag="ptr", bufs=1)
                    nc.tensor.transpose(
                        trp, pT[0:P_IN, bt * 128:(bt + 1) * 128], ident_sb)
                    nc.vector.tensor_copy(
                        osb[:, bt, lv * P_IN:(lv + 1) * P_IN], trp)

            def out_dma(g):
                o_r = out_d.ap().rearrange("p (t k) -> p t k", k=OUTW)
                nc.sync.dma_start(out=o_r[:, 4 * g:4 * g + 4],
                                  in_=osb[:, 4 * g:4 * g + 4])

            warm_i = [0]

            def warm(n):
                ps = ps_sm.tile([P_IN, 32], F32, tag="ptr", bufs=1,
                                name=f"warm{warm_i[0]}")
                warm_i[0] += 1
                for _ in range(n):
                    nc.tensor.matmul(ps, wsrc_sb, wsrc_sb,
                                     start=True, stop=True)

            # ================= emission schedule =================
            # PE order: enc(h0) | vals+scorer(h0) | exp-r-h0 chain | enc(h1)
            # | h0 mid block (DVE lrelu; transposes fill the DVE wait) |
            # exp-m-h0 chain | vals+scorer(h1) with h0 task/transpose hooks
            # | exposed h1 tail (Act-lrelu mid block, exps batched per
            # table visit, warm filler holds the PE pstate).
            encoder_half(0)
            lg0 = ps_lg.tile([P_IN, 512], F32, tag="lg", bufs=2, name="lg0")
            vals_scorer(0, lg0)
            softmax_chain(0, lg0, bsT_sb, 0, prT_sb)
            ml0 = ps_lg.tile([P_IN, 512], F32, tag="lg", bufs=2, name="ml0")

            def h0_mid_mms():
                for q in range(4):
                    mid_h_mms(0, q, act_lrelu=True)
                osb_transposes(0, prT_sb, 0)

            def h0_midlogit():
                mid_logit_mms(0, ml0)

            encoder_half(1, hooks={4: h0_mid_mms, 6: h0_midlogit})
            softmax_chain(0, ml0, b2cT_sb, 1, pmT_sb)

            lg1 = ps_lg.tile([P_IN, 512], F32, tag="lg", bufs=2, name="lg1")

            def h0_task():
                task_mms(0)

            def h0_trans_m():
                osb_transposes(0, pmT_sb, 1)

            def h0_y():
                y_tail(0, name="yl0")
                out_dma(0)

            vals_scorer(1, lg1, hooks={3: h0_task, 5: h0_trans_m, 6: h0_y})

            # ---------------- exposed h1 tail ----------------
            # root and mid chains run as two pipelined 256-col sub-chains:
            # mid-A matmuls start as soon as pred-A lands while chain-B is
            # still in the DVE queue.
            softmax_chain(1, lg1, bsT_sb, 0, prT_sb, ch=0)
            softmax_chain(1, lg1, bsT_sb, 0, prT_sb, ch=1)
            warm(8)
            for q in range(4):
                mid_h_mms(1, q, ch=0, act_lrelu=True)
            for q in range(4):
                mid_h_mms(1, q, ch=1, act_lrelu=True)
            ml1 = ps_lg.tile([P_IN, 512], F32, tag="lg", bufs=2, name="ml1")
            mid_logit_mms(1, ml1, ch=0)
            mid_logit_mms(1, ml1, ch=1)
            softmax_chain(1, ml1, b2cT_sb, 1, pmT_sb, ch=0)
            softmax_chain(1, ml1, b2cT_sb, 1, pmT_sb, ch=1)
            osb_transposes(1, prT_sb, 0)
            warm(6)
            task_mms(1, ch=0)
            task_mms(1, ch=1)
            warm(3)
            y_tail(1, ch=0, name="yl1a")
            osb_transposes(1, pmT_sb, 1, ch=0)
            o_r = out_d.ap().rearrange("p (t k) -> p t k", k=OUTW)
            nc.sync.dma_start(out=o_r[:, 4:6], in_=osb[:, 4:6])
            y_tail(1, ch=1, name="yl1b")
            osb_transposes(1, pmT_sb, 1, ch=1)
            nc.sync.dma_start(out=o_r[:, 6:8], in_=osb[:, 6:8])

    nc.compile()
    return nc


def prep_weights(inp):
    """Host-side reformatting of (replicated) weights to device layouts."""
    f32 = np.float32
    fp8 = ml_dtypes.float8_e4m3

    def to_fp8(a):
        return np.clip(a, -240.0, 240.0).astype(fp8)

    W_enc = np.asarray(inp["W_enc"], f32)          # [2048, 1024]
    Wv = np.asarray(inp["Wv"], f32)                # [8, 1024, 256]
    Ws = np.asarray(inp["Ws"], f32)                # [8, 256, 4]
    W1c = np.asarray(inp["W1c"], f32)              # [8, 32, 64]
    W2c = np.asarray(inp["W2c"], f32)              # [8, 64, 4]
    W1y = np.asarray(inp["W1y"], f32)              # [32, 64]
    W2y = np.asarray(inp["W2y"], f32)              # [64, 4]
    b1c = np.asarray(inp["b1c"], f32)
    b1y = np.asarray(inp["b1y"], f32)
    b2y = np.asarray(inp["b2y"], f32)

    # wenc [p, ht, kt, c]
    wenc = (SW * W_enc).reshape(KT_IN, 128, KT_H, 128).transpose(1, 2, 0, 3)
    # wv [p, r, kt, oc]
    wv = (SW * Wv).reshape(N_ROOT, KT_H, 128, OV).transpose(2, 0, 1, 3)
    # ws_big [ch, 2r+ot, 4r+c]
    wsb = np.zeros((128, 16, 32), f32)
    for r in range(N_ROOT):
        for ot in range(2):
            wsb[:, 2 * r + ot, 4 * r:4 * r + 4] = \
                Ws[r, ot * 128:(ot + 1) * 128, :]
    # w1c pair layout [32, 4, 128] + b1c ones-row -> [33, 512]
    w1c_flat = W1c.transpose(1, 0, 2).reshape(P_IN, 512)
    w1c_aug = np.concatenate([w1c_flat, b1c.reshape(1, 512)], axis=0)
    # w2c_big [64s+h, q, 4(2q+s)+c]
    w2cb = np.zeros((128, 4, 32), f32)
    for q in range(4):
        for s in range(2):
            w2cb[64 * s:64 * s + 64, q, 4 * (2 * q + s):4 * (2 * q + s) + 4] \
                = W2c[2 * q + s]
    w1y_aug = np.concatenate([W1y, b1y.reshape(1, P_HID)], axis=0)
    w2y_aug = np.concatenate([W2y, b2y.reshape(1, CARD)], axis=0)
    # block-diagonal group-sum matrix
    g32 = np.kron(np.eye(8, dtype=f32), np.ones((4, 4), f32))

    bf16 = ml_dtypes.bfloat16
    # packed fp32 consts [128, 26]
    cstf = np.zeros((128, 26), f32)
    cstf[:, 0:8] = (SX * np.asarray(inp["b_enc"], f32)).reshape(KT_H, 128).T
    cstf[:, 8:24] = np.asarray(inp["bv"], f32).reshape(N_ROOT, 2, 128) \
        .transpose(2, 0, 1).reshape(128, 16)
    cstf[0:P_IN, 24] = np.asarray(inp["bs"], f32).reshape(P_IN)
    cstf[0:P_IN, 25] = np.asarray(inp["b2c"], f32).reshape(P_IN)
    # packed bf16 consts [128, 1284]
    cstb = np.zeros((128, 1284), f32)
    cstb[:, 0:512] = wsb.reshape(128, 512)
    cstb[:, 512:640] = w2cb.reshape(128, 128)
    cstb[0:P_IN + 1, 640:1152] = w1c_aug
    cstb[0:P_IN + 1, 1152:1216] = w1y_aug
    cstb[0:P_HID + 1, 1216:1220] = w2y_aug
    cstb[0:P_IN, 1220:1252] = np.eye(P_IN, dtype=f32)
    cstb[0:P_IN, 1252:1284] = g32
    wmap = {
        "wenc": np.ascontiguousarray(to_fp8(wenc).reshape(128, -1)),
        "wv": np.ascontiguousarray(to_fp8(wv).reshape(128, -1)),
        "cstf": np.ascontiguousarray(cstf),
        "cstb": np.ascontiguousarray(cstb, dtype=bf16),
    }
    return wmap


def make_in_maps(inp):
    f32 = np.float32
    fp8 = ml_dtypes.float8_e4m3
    bf16 = ml_dtypes.bfloat16
    wmap = prep_weights(inp)
    x = np.asarray(inp["x"], f32)
    lab = np.asarray(inp["c"], np.int32)
    msk = np.asarray(inp["intervention_index"], np.int32)

    # transposed one-hot / mask tensors, [4grp+c, b] per core
    iot = np.arange(CARD, dtype=np.int32)
    in_maps = []
    for i in range(N_CORES):
        m = dict(wmap)
        xc = x[i * BSH:(i + 1) * BSH]                     # [1024, 2048]
        xt = np.clip(SX * xc, -240.0, 240.0).astype(fp8)
        xt = xt.reshape(2, 2, 256, KT_IN, 128).transpose(4, 0, 1, 3, 2)
        m["xt"] = np.ascontiguousarray(xt.reshape(128, -1))
        lc = lab[i * BSH:(i + 1) * BSH]                   # [1024, 17]
        mc = msk[i * BSH:(i + 1) * BSH]
        # ohb[4r+c, b] = (lab[b, r] == c); cols 0:1024 root, 1024:2048 mid
        ohr = (lc[:, :8, None] == iot).transpose(1, 2, 0).reshape(P_IN, BSH)
        ohm = (lc[:, 8:16, None] == iot).transpose(1, 2, 0).reshape(P_IN, BSH)
        mrr = np.repeat(mc[:, :8].T, CARD, axis=0)        # [32, 1024]
        mmm = np.repeat(mc[:, 8:16].T, CARD, axis=0)
        m["ohb"] = np.ascontiguousarray(
            np.concatenate([ohr, ohm], axis=1).astype(bf16))
        m["mb"] = np.ascontiguousarray(
            np.concatenate([mrr, mmm], axis=1).astype(np.uint8))
        in_maps.append(m)
    return in_maps


def unshard_out(res_out):
    """[128, 8*68] per-core DRAM layout -> [BSH, 17, 4]."""
    a = np.asarray(res_out, np.float32).reshape(128, BSH // 128, 17, CARD)
    return np.ascontiguousarray(a.transpose(1, 0, 2, 3)).reshape(
        BSH, 17, CARD)


_NC_CACHE = {}


def _get_nc():
    key = SIM_SAFE
    if key not in _NC_CACHE:
        _NC_CACHE[key] = build_program()
    return _NC_CACHE[key]


def kernel(**inputs):
    from concourse.bass_utils import run_bass_kernel_spmd

    nc = _get_nc()
    in_maps = make_in_maps(inputs)
    res = run_bass_kernel_spmd(nc, in_maps, list(range(N_CORES)))
    outs = [unshard_out(res.results[i]["out"]) for i in range(N_CORES)]
    return np.concatenate(outs, axis=0)


# revision 54
# speedup vs baseline: 1.0173x; 1.0173x over previous
"""Trainium2 Bass kernel for nn_C2BM_30537217474758 (gnn_message_passing).

Concept-bottleneck model:
  x_enc = lrelu(x @ W_enc + b_enc)                         [B, 1024]
  vals  = lrelu(einsum('bi,rio->bro', x_enc, Wv) + bv)     [B, 8, 256]
  p_root = softmax(einsum('bro,roc->brc', vals, Ws) + bs)  [B, 8, 4]
  p_root = intervene(p_root, c[:, :8], ii[:, :8])
  h     = lrelu(einsum('bp,nph->bnh', p_root.flat, W1c) + b1c)
  p_mid = softmax(einsum('bnh,nhc->bnc', h, W2c) + b2c); intervene
  y     = softmax(lrelu(p_mid.flat @ W1y + b1y) @ W2y + b2y)
  out   = concat([p_root, p_mid, y[:, None]], axis=1)      [B, 17, 4]

Strategy: pure data-parallel over 8 NeuronCores (batch shard 1024/core),
weights replicated.  The two large GEMMs (encoder and value-embedding,
~4.3 GFLOP each per core) run in fp8(e4m3) with DoubleRow perf mode (2x PE
throughput, fp32 PSUM accumulation); weights and x are pre-scaled on the
host (x*32, W*256) so fp8 quantization happens in the normal range, and
the scales are divided back out in the activation (lrelu is positively
homogeneous).  x is transposed and cast on the HOST, so the kernel does
zero on-chip transposition of x.

The scorer and mid/task propagators produce logits directly in TRANSPOSED
layout [32 = 8grp x 4card, batch] by using zero-padded block stationary
matrices, so softmax group sums become one tiny block-diagonal matmul and
the resulting probability tensor feeds the next propagator GEMM with no
transpose on the critical path.  Intervention one-hots/masks are
precomputed on the host in the same transposed layout.  Output staging
[batch, 68] is produced by small PE transposes off the critical path, and
the final DRAM output is [128, 8*68] per core, unsharded on the host.

Batch is processed in two 512-row halves so each half's softmax ->
propagator -> task tail (DVE/ACT latency chains) hides under the other
half's GEMMs.
"""

import os
import sys

try:
    import concourse  # noqa: F401
except ImportError:
    sys.path.insert(0, "/opt/trn_rl_repo")

import numpy as np
import ml_dtypes

import concourse.bacc as bacc
import concourse.tile as tile
from concourse import mybir

# ---------------- problem constants (hardcoded per contract) ----------------
B, D_IN, D_H = 8192, 2048, 1024
N_ROOT, N_MID, CARD, CHS = 8, 8, 4, 64
OV = CARD * CHS           # 256  value-embedding width per root
P_IN = N_ROOT * CARD      # 32
P_HID = 2 * P_IN          # 64
N_CORES = 8
BSH = B // N_CORES        # 1024 batch rows per core
KT_IN = D_IN // 128       # 16 contraction tiles for encoder
KT_H = D_H // 128         # 8 contraction tiles for Wv
OUTW = 17 * CARD          # 68 output cols per row

F32 = mybir.dt.float32
I32 = mybir.dt.int32
U8 = mybir.dt.uint8
BF16 = mybir.dt.bfloat16
FP8 = mybir.dt.float8e4
AF = mybir.ActivationFunctionType
ALU = mybir.AluOpType
AX = mybir.AxisListType
DR = mybir.MatmulPerfMode.DoubleRow

LRELU_ALPHA = 0.01
# host-side pre-scales so fp8 values land in the normal range
SX = 32.0                 # x and x_enc scale
SW = 256.0                # W_enc / Wv scale
# CoreSim does not implement Lrelu/Prelu; BASS_SIM_SAFE=1 swaps in Relu so
# the rest of the program can be validated in simulation.  On hardware we
# use Prelu (identical to leaky-relu via the alpha operand): it lives in
# the same activation-table set as Exp ('exp_and_others'), so the Act
# engine never reloads tables between lrelu and softmax work.
SIM_SAFE = os.environ.get("BASS_SIM_SAFE") == "1"
ACT_LRELU = AF.Relu if SIM_SAFE else AF.Prelu


def build_program():
    """Emit the per-core Bass program (identical on all 8 cores)."""
    nc = bacc.Bacc("TRN2", target_bir_lowering=False, debug=False,
                   num_devices=N_CORES)

    # ------------- DRAM I/O (all host-prepped layouts) -------------
    # xt: [p, half, chunk, kt, b] = 32*x[g*512+c*256+b, kt*128+p] in fp8
    xt_d = nc.dram_tensor("xt", [128, 2 * 2 * KT_IN * 256], FP8,
                          kind="ExternalInput")
    # wenc: [p, ht, kt, c] = 256*W_enc[kt*128+p, ht*128+c]
    wenc_d = nc.dram_tensor("wenc", [128, KT_H * KT_IN * 128], FP8,
                            kind="ExternalInput")
    # wv: [p, r, kt, oc] = 256*Wv[r, kt*128+p, oc]
    wv_d = nc.dram_tensor("wv", [128, N_ROOT * KT_H * OV], FP8,
                          kind="ExternalInput")
    # packed fp32 constants: benc(0:8) | bv(8:24) | bsT col 24 | b2cT col 25
    cstf_d = nc.dram_tensor("cstf", [128, 26], F32, kind="ExternalInput")
    # packed bf16 constants: ws_big [ch, 2r+ot, 4r+c] (cols 0:512) |
    # w2c_big [64s+h, q, 4(2q+s)+c] (512:640) | w1c+b1c [33, 4, 128]
    # (640:1152) | w1y+b1y [33, 64] (1152:1216) | w2y+b2y [65, 4]
    # (1216:1220) | ident32 (1220:1252) | g32 (1252:1284)
    cstb_d = nc.dram_tensor("cstb", [128, 1284], BF16, kind="ExternalInput")
    # transposed one-hots (bf16) and masks (u8): [4g+c | 4n+c, b];
    # cols 0:1024 = root level, 1024:2048 = mid level
    ohb_d = nc.dram_tensor("ohb", [P_IN, 2 * BSH], BF16, kind="ExternalInput")
    mb_d = nc.dram_tensor("mb", [P_IN, 2 * BSH], U8, kind="ExternalInput")
    # out: [p, bt, 68]
    out_d = nc.dram_tensor("out", [128, (BSH // 128) * OUTW], F32,
                           kind="ExternalOutput")

    with tile.TileContext(nc) as tc:
        with (
            tc.tile_pool(name="persist", bufs=1) as persist,
            tc.tile_pool(name="vals", bufs=3) as vals_pool,
            tc.tile_pool(name="tmp", bufs=2) as tmp_pool,
            tc.tile_pool(name="ps_mm", bufs=3, space="PSUM") as ps_mm,
            tc.tile_pool(name="ps_lg", bufs=1, space="PSUM") as ps_lg,
            tc.tile_pool(name="ps_sm", bufs=2, space="PSUM") as ps_sm,
        ):
            # -------- DMA order: x h0 + wenc ht0 gate the encoder ----------
            # SWDGE ring: xt halves then wv (needed from ~t+20us).
            # SP ring: wenc ht0, fp32 consts (gate the first act), rest of
            # wenc, packed bf16 consts, one-hots/masks.
            xt_sb = persist.tile([128, 2, 2, KT_IN, 256], FP8)
            xt_r = xt_d.ap().rearrange("p (g c k b) -> p g c k b",
                                       g=2, c=2, b=256)
            wenc_sb = persist.tile([128, KT_H, KT_IN, 128], FP8)
            wenc_r = wenc_d.ap().rearrange("p (h k c) -> p h k c",
                                           h=KT_H, c=128)
            for k4 in range(4):
                nc.gpsimd.dma_start(out=xt_sb[:, 0, 0, 4 * k4:4 * k4 + 4],
                                    in_=xt_r[:, 0, 0, 4 * k4:4 * k4 + 4])
                nc.sync.dma_start(out=wenc_sb[:, 0, 4 * k4:4 * k4 + 4],
                                  in_=wenc_r[:, 0, 4 * k4:4 * k4 + 4])
            nc.gpsimd.dma_start(out=xt_sb[:, 0, 1], in_=xt_r[:, 0, 1])
            nc.sync.dma_start(out=wenc_sb[:, 1], in_=wenc_r[:, 1])
            cstf_sb = persist.tile([128, 26], F32)
            nc.sync.dma_start(out=cstf_sb, in_=cstf_d.ap())
            benc_sb = cstf_sb[:, 0:8]
            bv_sb = cstf_sb[:, 8:24]
            bsT_sb = cstf_sb[0:P_IN, 24:25]
            b2cT_sb = cstf_sb[0:P_IN, 25:26]
            wv_sb = persist.tile([128, N_ROOT, KT_H, OV], FP8)
            wv_r = wv_d.ap().rearrange("p (r k o) -> p r k o",
                                       r=N_ROOT, o=OV)
            for r in range(N_ROOT):
                nc.gpsimd.dma_start(out=wv_sb[:, r], in_=wv_r[:, r])
            nc.gpsimd.dma_start(out=xt_sb[:, 1, 0], in_=xt_r[:, 1, 0])
            nc.gpsimd.dma_start(out=xt_sb[:, 1, 1], in_=xt_r[:, 1, 1])
            for ht in range(2, KT_H):
                nc.sync.dma_start(out=wenc_sb[:, ht], in_=wenc_r[:, ht])

            # packed bf16 constants (one DMA): wsb | w2cb | w1c | w1y | w2y
            # | ident | g32
            cstb_sb = persist.tile([128, 1284], BF16)
            nc.sync.dma_start(out=cstb_sb, in_=cstb_d.ap())
            wsb_sb = cstb_sb[:, 0:512].rearrange("p (q c) -> p q c", c=32)
            w2cb_sb = cstb_sb[:, 512:640].rearrange("p (q c) -> p q c", c=32)
            w1c_sb = cstb_sb[0:P_IN + 1, 640:1152].rearrange(
                "p (q m) -> p q m", m=128)
            w1y_sb = cstb_sb[0:P_IN + 1, 1152:1216]
            w2y_sb = cstb_sb[0:P_HID + 1, 1216:1220]
            ident_sb = cstb_sb[0:P_IN, 1220:1252]
            g32_sb = cstb_sb[0:P_IN, 1252:1284]

            ohb_sb = persist.tile([P_IN, 2 * BSH], BF16)
            nc.sync.dma_start(out=ohb_sb, in_=ohb_d.ap())
            mb_sb = persist.tile([P_IN, 2 * BSH], U8)
            nc.sync.dma_start(out=mb_sb, in_=mb_d.ap())

            # warm-up matmul source: memset, so PE filler never waits DMA
            wsrc_sb = persist.tile([P_IN, P_IN], BF16)
            nc.vector.memset(wsrc_sb, 1.0)

            # ---------------- persistent activations ----------------
            xenc_sb = persist.tile([128, KT_H, BSH], FP8)   # 32*x_encT
            prT_sb = persist.tile([P_IN + 1, BSH], BF16)    # row 32 = ones
            nc.vector.memset(prT_sb[P_IN:P_IN + 1, :], 1.0)
            pmT_sb = persist.tile([P_IN + 1, BSH], BF16)
            nc.vector.memset(pmT_sb[P_IN:P_IN + 1, :], 1.0)
            hyT_sb = persist.tile([P_HID + 1, BSH], BF16)   # row 64 = ones
            nc.vector.memset(hyT_sb[P_HID:P_HID + 1, :], 1.0)
            hT_sb = persist.tile([128, 4, BSH], BF16)  # [2 mids x 64h, b]
            osb = persist.tile([128, BSH // 128, OUTW], F32)

            # ---------------- encoder GEMM -> x_encT (fp8) ----------------
            def encoder_half(g, hooks=None):
                hooks = hooks or {}
                for ht in range(KT_H):
                    ps = ps_mm.tile([128, 512], F32, tag="mm")
                    for c in range(2):
                        for j in range(KT_IN // 2):
                            nc.tensor.matmul(
                                ps[:, c * 256:(c + 1) * 256],
                                wenc_sb[:, ht, 2 * j:2 * j + 2, :],
                                xt_sb[:, g, c, 2 * j:2 * j + 2, :],
                                start=(j == 0), stop=(j == KT_IN // 2 - 1),
                                perf_mode=DR, skip_group_check=(c == 1))
                    nc.scalar.activation(
                        xenc_sb[:, ht, g * 512:(g + 1) * 512], ps,
                        ACT_LRELU, bias=benc_sb[:, ht:ht + 1],
                        scale=float(SX / (SX * SW)), alpha=LRELU_ALPHA)
                    if ht in hooks:
                        hooks[ht]()

            def win(g, ch):
                """column window: absolute start, width for (half, chunk)."""
                if ch is None:
                    return g * 512, 512
                return g * 512 + ch * 256, 256

            # ------------- per-root value GEMM + scorer (one window) --------
            def vals_scorer(g, lg, ch=None, hooks=None):
                """Value embeddings + scorer over one column window;
                logitsT into lg[:, window].  The scorer for root r is
                emitted after root r+1's value GEMMs so the PE never
                stalls on the vals activation (a stall resets the PE
                pstate ramp).  hooks[r] emits extra (tail) work after
                root r's GEMMs."""
                c0, n = win(g, ch)
                lgw = lg if ch is None else lg[:, ch * 256:(ch + 1) * 256]
                hooks = hooks or {}
                vts = {}

                def scorer(r):
                    for ot in range(2):
                        nc.tensor.matmul(
                            lgw, wsb_sb[:, 2 * r + ot, :], vts[r][:, ot, :],
                            start=(r == 0 and ot == 0),
                            stop=(r == N_ROOT - 1 and ot == 1),
                            skip_group_check=(ch == 1))

                shared = [None]
                for r in range(N_ROOT):
                    vt = vals_pool.tile([128, 2, n], BF16, tag="vals")
                    vts[r] = vt
                    for ot in range(2):
                        if n == 512:
                            ps = ps_mm.tile([128, 512], F32, tag="mm")
                            reg = 0
                        else:
                            # pair two 256-wide chunks per [128,512] tile
                            reg = (2 * r + ot) % 2
                            if reg == 0:
                                shared[0] = ps_mm.tile([128, 512], F32,
                                                       tag="mm",
                                                       name="vshare")
                            ps = shared[0][:, reg * 256:(reg + 1) * 256]
                        for c in range(n // 256):
                            nc_ps = ps[:, c * 256:(c + 1) * 256] \
                                if n == 512 else ps
                            for j in range(KT_H // 2):
                                nc.tensor.matmul(
                                    nc_ps,
                                    wv_sb[:, r, 2 * j:2 * j + 2,
                                          ot * 128:(ot + 1) * 128],
                                    xenc_sb[:, 2 * j:2 * j + 2,
                                            c0 + c * 256:c0 + (c + 1) * 256],
                                    start=(j == 0), stop=(j == KT_H // 2 - 1),
                                    perf_mode=DR,
                                    skip_group_check=(c == 1 or reg == 1))
                        nc.scalar.activation(
                            vt[:, ot, :], ps, ACT_LRELU,
                            bias=bv_sb[:, 2 * r + ot:2 * r + ot + 1],
                            scale=float(1.0 / (SX * SW)), alpha=LRELU_ALPHA)
                    if r >= 1:
                        scorer(r - 1)
                    if r in hooks:
                        hooks[r]()
                scorer(N_ROOT - 1)

            # ------------- transposed softmax + intervention tail ----------
            def softmax_chain(g, lg, bias, lv, pT, ch=None, sc=1.0):
                """softmax+intervene on logitsT window of lg (PSUM);
                probs -> pT[0:32, window] (bf16)."""
                c0, n = win(g, ch)
                lgw = lg if ch is None else lg[:, ch * 256:(ch + 1) * 256]
                cols = slice(c0, c0 + n)
                pcols = slice(lv * BSH + c0, lv * BSH + c0 + n)
                e = tmp_pool.tile([P_IN, n], BF16, tag="e", bufs=3)
                nc.scalar.activation(e, lgw, AF.Exp, bias=bias, scale=sc)
                sm = ps_lg.tile([P_IN, n], F32, tag="lg", bufs=2, name="sums")
                nc.tensor.matmul(sm, g32_sb, e, start=True, stop=True)
                rcp = tmp_pool.tile([P_IN, n], F32, tag="rcp", bufs=2)
                nc.vector.reciprocal_approx_fast(rcp, sm)
                nc.vector.tensor_tensor(pT[0:P_IN, cols], e, rcp, op=ALU.mult)
                nc.vector.copy_predicated(pT[0:P_IN, cols], mb_sb[:, pcols],
                                          ohb_sb[:, pcols])

            def dve_lrelu(dst, ps, n, tag):
                if SIM_SAFE:
                    nc.vector.tensor_scalar(dst, ps, 0.0, None, op0=ALU.max)
                else:
                    t = tmp_pool.tile([128, n], BF16, tag=tag, bufs=2)
                    tt = t[0:dst.partition_size(), :]
                    nc.vector.tensor_scalar(tt, ps, LRELU_ALPHA, None,
                                            op0=ALU.mult)
                    nc.vector.tensor_tensor(dst, ps, tt, op=ALU.max)

            def mid_h_mms(g, q, ch=None, act_lrelu=False):
                c0, n = win(g, ch)
                ps = ps_mm.tile([128, n], F32, tag="mmh", bufs=2)
                nc.tensor.matmul(ps, w1c_sb[:, q, :], prT_sb[:, c0:c0 + n],
                                 start=True, stop=True)
                dst = hT_sb[:, q, c0:c0 + n]
                if act_lrelu and q % 2 == 0 and not SIM_SAFE:
                    nc.scalar.activation(dst, ps, ACT_LRELU,
                                         alpha=LRELU_ALPHA)
                else:
                    dve_lrelu(dst, ps, n, "lr")

            def mid_logit_mms(g, ml, ch=None):
                c0, n = win(g, ch)
                mlw = ml if ch is None else ml[:, ch * 256:(ch + 1) * 256]
                for q in range(4):
                    nc.tensor.matmul(
                        mlw, w2cb_sb[:, q, :], hT_sb[:, q, c0:c0 + n],
                        start=(q == 0), stop=(q == 3),
                        skip_group_check=(ch == 1))

            def bts_of(g, ch):
                if ch is None:
                    return list(range(4 * g, 4 * g + 4))
                return [4 * g + 2 * ch, 4 * g + 2 * ch + 1]

            def task_mms(g, ch=None, act_lrelu=False):
                c0, n = win(g, ch)
                if n == 512:
                    ps = ps_mm.tile([P_HID, 512], F32, tag="mm")
                else:
                    ps = ps_mm.tile([P_HID, 256], F32, tag="mmh", bufs=2)
                nc.tensor.matmul(ps, w1y_sb, pmT_sb[:, c0:c0 + n],
                                 start=True, stop=True)
                dst = hyT_sb[0:P_HID, c0:c0 + n]
                if act_lrelu and not SIM_SAFE:
                    nc.scalar.activation(dst, ps, ACT_LRELU,
                                         alpha=LRELU_ALPHA)
                else:
                    dve_lrelu(dst, ps, n, "lry")

            def y_tail(g, ch=None, name="yl"):
                bts = bts_of(g, ch)
                nb = len(bts)
                yl = ps_sm.tile([128, 4 * nb], F32, tag="ptr", bufs=1,
                                name=name)
                for i, bt in enumerate(bts):
                    nc.tensor.matmul(
                        yl[:, i * 4:(i + 1) * 4],
                        hyT_sb[:, bt * 128:(bt + 1) * 128], w2y_sb,
                        start=True, stop=True, skip_group_check=True)
                e4 = tmp_pool.tile([128, 4 * nb], F32, tag="e4")
                nc.scalar.activation(e4, yl[:, 0:4 * nb], AF.Exp)
                s1 = tmp_pool.tile([128, nb], F32, tag="s1")
                nc.vector.reduce_sum(
                    s1, e4.rearrange("p (b c) -> p b c", c=CARD), axis=AX.X)
                r1 = tmp_pool.tile([128, nb], F32, tag="r1")
                nc.vector.reciprocal(r1, s1)
                nc.vector.tensor_tensor(
                    osb[:, bts[0]:bts[0] + nb, 16 * CARD:17 * CARD],
                    e4.rearrange("p (b c) -> p b c", c=CARD),
                    r1.unsqueeze(2).broadcast_to([128, nb, CARD]),
                    op=ALU.mult)

            def osb_transposes(g, pT, lv, ch=None):
                """pT[0:32, window] -> osb[:, bt, lv*32:(lv+1)*32]."""
                for bt in bts_of(g, ch):
                    trp = ps_sm.tile([128, P_IN], BF16, tag="ptr", bufs=1)
                    nc.tensor.transpose(
                        trp, pT[0:P_IN, bt * 128:(bt + 1) * 128], ident_sb)
                    nc.vector.tensor_copy(
                        osb[:, bt, lv * P_IN:(lv + 1) * P_IN], trp)

            def out_dma(g):
                o_r = out_d.ap().rearrange("p (t k) -> p t k", k=OUTW)
                nc.sync.dma_start(out=o_r[:, 4 * g:4 * g + 4],
                                  in_=osb[:, 4 * g:4 * g + 4])

            warm_i = [0]

            def warm(n):
                ps = ps_sm.tile([P_IN, 32], F32, tag="ptr", bufs=1,
                                name=f"warm{warm_i[0]}")
                warm_i[0] += 1
                for _ in range(n):
                    nc.tensor.matmul(ps, wsrc_sb, wsrc_sb,
                                     start=True, stop=True)

            # ================= emission schedule =================
            # PE order: enc(h0) | vals+scorer(h0) | exp-r-h0 chain | enc(h1)
            # | h0 mid block (DVE lrelu; transposes fill the DVE wait) |
            # exp-m-h0 chain | vals+scorer(h1) with h0 task/transpose hooks
            # | exposed h1 tail (Act-lrelu mid block, exps batched per
            # table visit, warm filler holds the PE pstate).
            encoder_half(0)
            lg0 = ps_lg.tile([P_IN, 512], F32, tag="lg", bufs=2, name="lg0")
            vals_scorer(0, lg0)
            softmax_chain(0, lg0, bsT_sb, 0, prT_sb)
            ml0 = ps_lg.tile([P_IN, 512], F32, tag="lg", bufs=2, name="ml0")

            def h0_mid_mms():
                for q in range(4):
                    mid_h_mms(0, q, act_lrelu=True)
                osb_transposes(0, prT_sb, 0)

            def h0_midlogit():
                mid_logit_mms(0, ml0)

            encoder_half(1, hooks={4: h0_mid_mms, 6: h0_midlogit})
            softmax_chain(0, ml0, b2cT_sb, 1, pmT_sb)

            lg1 = ps_lg.tile([P_IN, 512], F32, tag="lg", bufs=2, name="lg1")

            def h0_task():
                task_mms(0)

            def h0_trans_m():
                osb_transposes(0, pmT_sb, 1)

            def h0_y():
                y_tail(0, name="yl0")
                out_dma(0)

            vals_scorer(1, lg1, hooks={3: h0_task, 5: h0_trans_m, 6: h0_y})

            # ---------------- exposed h1 tail ----------------
            # root and mid chains run as two pipelined 256-col sub-chains:
            # mid-A matmuls start as soon as pred-A lands while chain-B is
            # still in the DVE queue.
            softmax_chain(1, lg1, bsT_sb, 0, prT_sb, ch=0)
            softmax_chain(1, lg1, bsT_sb, 0, prT_sb, ch=1)
            warm(8)
            for q in range(4):
                mid_h_mms(1, q, ch=0, act_lrelu=True)
            for q in range(4):
                mid_h_mms(1, q, ch=1, act_lrelu=True)
            ml1 = ps_lg.tile([P_IN, 512], F32, tag="lg", bufs=2, name="ml1")
            mid_logit_mms(1, ml1, ch=0)
            mid_logit_mms(1, ml1, ch=1)
            softmax_chain(1, ml1, b2cT_sb, 1, pmT_sb, ch=0)
            softmax_chain(1, ml1, b2cT_sb, 1, pmT_sb, ch=1)
            osb_transposes(1, prT_sb, 0)
            warm(6)
            task_mms(1, ch=0)
            task_mms(1, ch=1)
            warm(3)
            y_tail(1, ch=0, name="yl1a")
            osb_transposes(1, pmT_sb, 1, ch=0)
            o_r = out_d.ap().rearrange("p (t k) -> p t k", k=OUTW)
            nc.sync.dma_start(out=o_r[:, 4:6], in_=osb[:, 4:6])
            y_tail(1, ch=1, name="yl1b")
            osb_transposes(1, pmT_sb, 1, ch=1)
            nc.sync.dma_start(out=o_r[:, 6:8], in_=osb[:, 6:8])

    nc.compile()
    return nc


def prep_weights(inp):
    """Host-side reformatting of (replicated) weights to device layouts."""
    f32 = np.float32
    fp8 = ml_dtypes.float8_e4m3

    def to_fp8(a):
        return np.clip(a, -240.0, 240.0).astype(fp8)

    W_enc = np.asarray(inp["W_enc"], f32)          # [2048, 1024]
    Wv = np.asarray(inp["Wv"], f32)                # [8, 1024, 256]
    Ws = np.asarray(inp["Ws"], f32)                # [8, 256, 4]
    W1c = np.asarray(inp["W1c"], f32)              # [8, 32, 64]
    W2c = np.asarray(inp["W2c"], f32)              # [8, 64, 4]
    W1y = np.asarray(inp["W1y"], f32)              # [32, 64]
    W2y = np.asarray(inp["W2y"], f32)              # [64, 4]
    b1c = np.asarray(inp["b1c"], f32)
    b1y = np.asarray(inp["b1y"], f32)
    b2y = np.asarray(inp["b2y"], f32)

    # wenc [p, ht, kt, c]
    wenc = (SW * W_enc).reshape(KT_IN, 128, KT_H, 128).transpose(1, 2, 0, 3)
    # wv [p, r, kt, oc]
    wv = (SW * Wv).reshape(N_ROOT, KT_H, 128, OV).transpose(2, 0, 1, 3)
    # ws_big [ch, 2r+ot, 4r+c]
    wsb = np.zeros((128, 16, 32), f32)
    for r in range(N_ROOT):
        for ot in range(2):
            wsb[:, 2 * r + ot, 4 * r:4 * r + 4] = \
                Ws[r, ot * 128:(ot + 1) * 128, :]
    # w1c pair layout [32, 4, 128] + b1c ones-row -> [33, 512]
    w1c_flat = W1c.transpose(1, 0, 2).reshape(P_IN, 512)
    w1c_aug = np.concatenate([w1c_flat, b1c.reshape(1, 512)], axis=0)
    # w2c_big [64s+h, q, 4(2q+s)+c]
    w2cb = np.zeros((128, 4, 32), f32)
    for q in range(4):
        for s in range(2):
            w2cb[64 * s:64 * s + 64, q, 4 * (2 * q + s):4 * (2 * q + s) + 4] \
                = W2c[2 * q + s]
    w1y_aug = np.concatenate([W1y, b1y.reshape(1, P_HID)], axis=0)
    w2y_aug = np.concatenate([W2y, b2y.reshape(1, CARD)], axis=0)
    # block-diagonal group-sum matrix
    g32 = np.kron(np.eye(8, dtype=f32), np.ones((4, 4), f32))

    bf16 = ml_dtypes.bfloat16
    # packed fp32 consts [128, 26]
    cstf = np.zeros((128, 26), f32)
    cstf[:, 0:8] = (SX * np.asarray(inp["b_enc"], f32)).reshape(KT_H, 128).T
    cstf[:, 8:24] = np.asarray(inp["bv"], f32).reshape(N_ROOT, 2, 128) \
        .transpose(2, 0, 1).reshape(128, 16)
    cstf[0:P_IN, 24] = np.asarray(inp["bs"], f32).reshape(P_IN)
    cstf[0:P_IN, 25] = np.asarray(inp["b2c"], f32).reshape(P_IN)
    # packed bf16 consts [128, 1284]
    cstb = np.zeros((128, 1284), f32)
    cstb[:, 0:512] = wsb.reshape(128, 512)
    cstb[:, 512:640] = w2cb.reshape(128, 128)
    cstb[0:P_IN + 1, 640:1152] = w1c_aug
    cstb[0:P_IN + 1, 1152:1216] = w1y_aug
    cstb[0:P_HID + 1, 1216:1220] = w2y_aug
    cstb[0:P_IN, 1220:1252] = np.eye(P_IN, dtype=f32)
    cstb[0:P_IN, 1252:1284] = g32
    wmap = {
        "wenc": np.ascontiguousarray(to_fp8(wenc).reshape(128, -1)),
        "wv": np.ascontiguousarray(to_fp8(wv).reshape(128, -1)),
        "cstf": np.ascontiguousarray(cstf),
        "cstb": np.ascontiguousarray(cstb, dtype=bf16),
    }
    return wmap


def make_in_maps(inp):
    f32 = np.float32
    fp8 = ml_dtypes.float8_e4m3
    bf16 = ml_dtypes.bfloat16
    wmap = prep_weights(inp)
    x = np.asarray(inp["x"], f32)
    lab = np.asarray(inp["c"], np.int32)
    msk = np.asarray(inp["intervention_index"], np.int32)

    # transposed one-hot / mask tensors, [4grp+c, b] per core
    iot = np.arange(CARD, dtype=np.int32)
    in_maps = []
    for i in range(N_CORES):
        m = dict(wmap)
        xc = x[i * BSH:(i + 1) * BSH]                     # [1024, 2048]
        xt = np.clip(SX * xc, -240.0, 240.0).astype(fp8)
        xt = xt.reshape(2, 2, 256, KT_IN, 128).transpose(4, 0, 1, 3, 2)
        m["xt"] = np.ascontiguousarray(xt.reshape(128, -1))
        lc = lab[i * BSH:(i + 1) * BSH]                   # [1024, 17]
        mc = msk[i * BSH:(i + 1) * BSH]
        # ohb[4r+c, b] = (lab[b, r] == c); cols 0:1024 root, 1024:2048 mid
        ohr = (lc[:, :8, None] == iot).transpose(1, 2, 0).reshape(P_IN, BSH)
        ohm = (lc[:, 8:16, None] == iot).transpose(1, 2, 0).reshape(P_IN, BSH)
        mrr = np.repeat(mc[:, :8].T, CARD, axis=0)        # [32, 1024]
        mmm = np.repeat(mc[:, 8:16].T, CARD, axis=0)
        m["ohb"] = np.ascontiguousarray(
            np.concatenate([ohr, ohm], axis=1).astype(bf16))
        m["mb"] = np.ascontiguousarray(
            np.concatenate([mrr, mmm], axis=1).astype(np.uint8))
        in_maps.append(m)
    return in_maps


def unshard_out(res_out):
    """[128, 8*68] per-core DRAM layout -> [BSH, 17, 4]."""
    a = np.asarray(res_out, np.float32).reshape(128, BSH // 128, 17, CARD)
    return np.ascontiguousarray(a.transpose(1, 0, 2, 3)).reshape(
        BSH, 17, CARD)


_NC_CACHE = {}


def _get_nc():
    key = SIM_SAFE
    if key not in _NC_CACHE:
        _NC_CACHE[key] = build_program()
    return _NC_CACHE[key]


def kernel(**inputs):
    from concourse.bass_utils import run_bass_kernel_spmd

    nc = _get_nc()
    in_maps = make_in_maps(inputs)
    res = run_bass_kernel_spmd(nc, in_maps, list(range(N_CORES)))
    outs = [unshard_out(res.results[i]["out"]) for i in range(N_CORES)]
    return np.concatenate(outs, axis=0)
